# revision 3
# baseline (speedup 1.0000x reference)
"""DenseExpert MoE kernel for 8x Trainium2 NeuronCores — v3.1.

Math: r[b,u] = elu( sum_e g[b,e] * (x[b,:] @ alpha[e,u,:]) + (g @ beta)[b,u] )
x [4096,512] f32, g [4096,8] f32, alpha [8,512,512] f32, beta [8,512] f32
-> out [4096,512] f32.

Data-parallel over batch across 8 cores; alpha replicated. Host-side shard
prep stages device-friendly bf16 layouts (data movement/rounding only, all
arithmetic stays on-device):
  xT   [128, KT, BT, 128]  x transposed (contraction dim on partitions)
  gbeta [E, BS+U]          g.T and beta concatenated
  gb   [E, 128, BS]        gate rows replicated across partitions
  alphaT [E, D, U]         alpha transposed
Device work per core: per batch tile bt one PSUM accumulator takes
  bias (gT.T @ beta, start=True) + sum_e sum_kt (g_e*x)T_kt @ alphaT_e_kt
with (g_e*x)T made by bf16 DVE tensor_tensor against the pre-broadcast gate
tile (327ns per tile, free-dim broadcast). No on-device transposes: the DMA
stream is pure copy mode (loads ordered on one SP FIFO = supply order, each
expert's gate tile riding just ahead of its alpha). Expert 0 runs in kt
blocks so its matmuls start on the first half of alpha0; experts 4-7 then
run tile-major so tiles finish accumulating ~4.5us apart — epilogues
(exp / min(exp-1,0) / max(r,0)+m / store) overlap the matmul stream; the
final tile accumulates in four quarter-width PSUM tiles closed piece-major
so only one quarter's chain trails the last matmul. A constant-fed warm-up
matmul chain ramps the PE clock to full speed before the first real matmul.
"""
import sys as _sys
for _p in ("/opt/trn_rl_repo", "/root/.axon_site/_ro/trn_rl_repo"):
    if _p not in _sys.path:
        _sys.path.append(_p)

import numpy as np

N_CORES = 8
B, D, U, E = 4096, 512, 512, 8
BS = B // N_CORES       # 512 tokens per core
BT = BS // 128          # 4 batch tiles per core
KT = D // 128           # 4 contraction blocks

N_WARM_CONST = 9        # constant-fed warm-up matmuls (256-col)
N_WARM_DATA = 0         # beta-fed warm-up matmuls (512-col) after bias

# (expert, tile) visit order: experts 0-3 expert-major (matches the alpha
# load pipeline), then each tile's experts 4-7 consecutively so tiles finish
# accumulating at ~21/25/28/33us — epilogues spread over the matmul stream
# instead of piling up after the last matmul.
PAIR_ORDER = [(e, bt) for e in range(4) for bt in range(BT)] + \
             [(e, bt) for bt in range(BT - 1) for e in range(4, 8)] + [(7, 3)]
LAST_OF = {bt: (7, bt) for bt in range(BT)}

_CACHE = {}


def _build_module():
    import concourse.tile as tile
    from concourse import bacc, mybir

    f32 = mybir.dt.float32
    bf16 = mybir.dt.bfloat16
    MULT = mybir.AluOpType.mult
    Exp = mybir.ActivationFunctionType.Exp
    Relu = mybir.ActivationFunctionType.Relu

    nc = bacc.Bacc("TRN2", target_bir_lowering=False, debug=False,
                   num_devices=N_CORES)
    xT_d = nc.dram_tensor("xT", [128, KT, BT, 128], bf16,
                          kind="ExternalInput").ap()
    gbeta_d = nc.dram_tensor("gbeta", [E, BS + U], bf16,
                             kind="ExternalInput").ap()
    gb_d = nc.dram_tensor("gb", [E, 128, BS], bf16, kind="ExternalInput").ap()
    aT_d = nc.dram_tensor("alphaT", [E, D, U], bf16,
                          kind="ExternalInput").ap()
    o_d = nc.dram_tensor("out", [BS, U], f32, kind="ExternalOutput").ap()
    o_r = o_d.rearrange("(bt p) u -> p bt u", p=128)

    with tile.TileContext(nc, trace_sim=True) as tc:
        with (
            tc.tile_pool(name="const", bufs=1) as cpool,
            tc.tile_pool(name="rps", bufs=1, space="PSUM") as rpool,
            tc.tile_pool(name="xe", bufs=8) as xepool,
            tc.tile_pool(name="work", bufs=10) as wpool,
        ):
            # ---- g/beta: one bf16 load via the gpsimd SWDGE generator
            # (parallel to the HWDGE chain that feeds the SP supply FIFO) ----
            gbeta_bf = cpool.tile([E, BS + U], bf16)
            nc.gpsimd.dma_start(gbeta_bf[:], gbeta_d)
            gT_bf = gbeta_bf[:, 0:BS]
            beta_sb = gbeta_bf[:, BS:BS + U]

            # ---- PE warm-up chain (no data deps); warms borrow r_ps0's
            # bank (start/stop groups complete before the bias re-opens it)
            ones_w = cpool.tile([1, 512], bf16)
            nc.vector.memset(ones_w[:], 1.0)
            r_ps0 = rpool.tile([128, U], f32, tag="r0", name="r_ps0")
            for i in range(N_WARM_CONST):
                nc.tensor.matmul(r_ps0[:, 0:256], lhsT=ones_w[0:1, 0:128],
                                 rhs=ones_w[0:1, 0:256], start=True, stop=True)

            # ---- HWDGE bf16 loads: one SP FIFO in exact consumption
            # order (the DMA device is serial, so FIFO order = supply
            # order); pure copy mode throughout ----
            gbs = cpool.tile([128, E, BT, 128], bf16)
            gb_r = gb_d.rearrange("e p (bt b) -> p e bt b", bt=BT)
            xT = cpool.tile([128, KT, BT, 128], bf16)
            aTs = []
            for e in range(E):
                aT_e = cpool.tile([128, KT, U], bf16, tag=f"aT{e}",
                                  name=f"aT{e}")
                aTs.append(aT_e)
            aT_r = aT_d.rearrange("e (kt p) u -> e p kt u", p=128)
            # a0 in kt halves: the kt01 block of e0's matmuls starts on the
            # first half
            nc.sync.dma_start(xT[:, 0:2], xT_d[:, 0:2])
            nc.sync.dma_start(gbs[:, 0:1, :, :], gb_r[:, 0:1])
            nc.sync.dma_start(aTs[0][:, 0:2, :], aT_r[0, :, 0:2, :])
            nc.sync.dma_start(xT[:, 2:4], xT_d[:, 2:4])
            nc.sync.dma_start(aTs[0][:, 2:4, :], aT_r[0, :, 2:4, :])
            # each expert's gate tile rides just ahead of its alpha
            for e in range(1, E):
                nc.sync.dma_start(gbs[:, e:e + 1, :, :], gb_r[:, e:e + 1])
                nc.sync.dma_start(aTs[e][:], aT_r[e])

            # ---- bias matmuls open each accumulator, then data warm-ups ----
            # bt3 gets two half-width accumulators so its final epilogue can
            # run in halves overlapped with the last matmuls
            r_pss = [r_ps0]
            for bt in range(1, BT - 1):
                r_ps = rpool.tile([128, U], f32, tag=f"r{bt}", name=f"r_ps{bt}")
                r_pss.append(r_ps)
            P3 = [(0, 128), (128, 256), (256, 384), (384, 512)]
            r3 = []
            for q, (lo, hi) in enumerate(P3):
                r_ps = rpool.tile([128, hi - lo], f32, tag=f"r3{q}",
                                  name=f"r_ps3{q}")
                r3.append(r_ps)
            for i in range(N_WARM_DATA):
                nc.tensor.matmul(r_ps0[:], lhsT=gT_bf[:, 0:128],
                                 rhs=beta_sb[:], start=True, stop=True)
            for bt in range(BT - 1):
                nc.tensor.matmul(r_pss[bt][:],
                                 lhsT=gT_bf[:, bt * 128:(bt + 1) * 128],
                                 rhs=beta_sb[:], start=True, stop=False)
            for q, (lo, hi) in enumerate(P3):
                nc.tensor.matmul(r3[q][:],
                                 lhsT=gT_bf[:, 3 * 128:4 * 128],
                                 rhs=beta_sb[:, lo:hi],
                                 start=True, stop=False)

            # ---- gate-scaled xT tiles ----
            def emit_scale(e, bt):
                xe = xepool.tile([128, KT, 128], bf16, tag="xe",
                                 name=f"xe_{e}_{bt}")
                nc.vector.tensor_tensor(
                    xe[:], xT[:, :, bt, :],
                    gbs[:, e, bt:bt + 1, :].broadcast_to([128, KT, 128]),
                    MULT)
                return xe

            def emit_mains(e, bt, xe, stop=False):
                if bt < BT - 1:
                    for kt in range(KT):
                        nc.tensor.matmul(
                            r_pss[bt][:], lhsT=xe[:, kt, :],
                            rhs=aTs[e][:, kt, :],
                            start=False, stop=(stop and kt == KT - 1))
                else:
                    for q, (lo, hi) in enumerate(P3):
                        for kt in range(KT):
                            nc.tensor.matmul(
                                r3[q][:], lhsT=xe[:, kt, :],
                                rhs=aTs[e][:, kt, lo:hi],
                                start=False, stop=(stop and kt == KT - 1))

            # ---- ELU + store epilogue ----
            def emit_epilogue(bt, lo, hi, store_eng, m_on_act=False):
                w = hi - lo
                if bt < BT - 1:
                    r_view = r_pss[bt][:, lo:hi]
                else:
                    r_view = r3[P3.index((lo, hi))][:]
                t_sb = wpool.tile([128, w], bf16, tag="t", name=f"t_{bt}_{lo}")
                nc.scalar.activation(t_sb[:], r_view, Exp)
                m_sb = wpool.tile([128, w], bf16, tag="m", name=f"m_{bt}_{lo}")
                if m_on_act:
                    # m' = relu(1 - t) on ACT right after the exp (no
                    # cross-engine hop); o = max(r,0) - m'
                    nc.scalar.activation(m_sb[:], t_sb[:], Relu,
                                         bias=1.0, scale=-1.0)
                    comb = mybir.AluOpType.subtract
                else:
                    # m = min(t - 1, 0) on DVE; o = max(r,0) + m
                    nc.vector.tensor_scalar(
                        out=m_sb[:], in0=t_sb[:], scalar1=-1.0, scalar2=0.0,
                        op0=mybir.AluOpType.add, op1=mybir.AluOpType.min)
                    comb = mybir.AluOpType.add
                o_sb = wpool.tile([128, w], f32, tag="o", name=f"o_{bt}_{lo}")
                nc.vector.scalar_tensor_tensor(
                    out=o_sb[:], in0=r_view, scalar=0.0, in1=m_sb[:],
                    op0=mybir.AluOpType.max, op1=comb)
                store_eng.dma_start(o_r[:, bt, lo:hi], o_sb[:])

            # expert 0 in kt blocks: kt01 over all tiles (unlocked by the
            # first half of a0), then kt23
            xe0s = []
            for bt in range(BT):
                xe = xepool.tile([128, KT, 128], bf16, tag="xe",
                                 name=f"xe_0_{bt}")
                nc.vector.tensor_tensor(
                    xe[:, 0:2, :], xT[:, 0:2, bt, :],
                    gbs[:, 0, bt:bt + 1, :].broadcast_to([128, 2, 128]),
                    MULT)
                xe0s.append(xe)
            for bt in range(BT):
                nc.vector.tensor_tensor(
                    xe0s[bt][:, 2:4, :], xT[:, 2:4, bt, :],
                    gbs[:, 0, bt:bt + 1, :].broadcast_to([128, 2, 128]),
                    MULT)

            def _e0_block(klo, bts):
                for bt in bts:
                    if bt < BT - 1:
                        for kt in (klo, klo + 1):
                            nc.tensor.matmul(
                                r_pss[bt][:], lhsT=xe0s[bt][:, kt, :],
                                rhs=aTs[0][:, kt, :], start=False, stop=False)
                    else:
                        for q, (lo, hi) in enumerate(P3):
                            for kt in (klo, klo + 1):
                                nc.tensor.matmul(
                                    r3[q][:], lhsT=xe0s[bt][:, kt, :],
                                    rhs=aTs[0][:, kt, lo:hi],
                                    start=False, stop=False)

            _e0_block(0, (0, 1, 2, 3))
            _e0_block(2, (0, 1, 2, 3))

            store_engs = {0: nc.sync, 1: nc.scalar, 2: nc.sync}
            for (e, bt) in PAIR_ORDER:
                if e == 0:
                    continue
                xe = emit_scale(e, bt)
                is_last = (LAST_OF[bt] == (e, bt))
                if not is_last:
                    emit_mains(e, bt, xe)
                    continue
                if bt < BT - 1:
                    emit_mains(e, bt, xe, stop=True)
                    emit_epilogue(bt, 0, U, store_engs[bt])
                else:
                    # final tile: piece-major over its last four experts so
                    # pieces close in a stagger and their epilogue chains
                    # overlap the remaining matmuls
                    q_engs = [nc.sync, nc.gpsimd, nc.sync, nc.sync]
                    xes = {7: xe}
                    for q, (lo, hi) in enumerate(P3):
                        for e2 in range(4, 8):
                            if e2 not in xes:
                                xes[e2] = emit_scale(e2, bt)
                            for kt in range(KT):
                                nc.tensor.matmul(
                                    r3[q][:], lhsT=xes[e2][:, kt, :],
                                    rhs=aTs[e2][:, kt, lo:hi],
                                    start=False,
                                    stop=(e2 == 7 and kt == KT - 1))
                        emit_epilogue(bt, lo, hi, q_engs[q])
    nc.compile()
    return nc


def get_module():
    if "nc" not in _CACHE:
        _CACHE["nc"] = _build_module()
    return _CACHE["nc"]


def make_in_maps(x, g, alpha, beta):
    import ml_dtypes
    bf = ml_dtypes.bfloat16
    aT_h = np.ascontiguousarray(alpha.transpose(0, 2, 1).astype(bf))
    in_maps = []
    for c in range(N_CORES):
        xc = x[c * BS:(c + 1) * BS]       # [BS, D]
        gc = g[c * BS:(c + 1) * BS]       # [BS, E]
        gcT = np.ascontiguousarray(gc.T)  # [E, BS]
        gcT_bf = gcT.astype(bf)
        xck = np.ascontiguousarray(
            xc.reshape(BT, 128, KT, 128).transpose(3, 2, 0, 1).astype(bf))
        in_maps.append({
            "xT": xck,                               # [128, KT, BT, 128]
            "gbeta": np.ascontiguousarray(np.concatenate(
                [gcT, beta], axis=1).astype(bf)),                   # [E, BS+U]
            "gb": np.ascontiguousarray(
                np.broadcast_to(gcT_bf[:, None, :], (E, 128, BS))),  # [E,128,BS]
            "alphaT": aT_h,
        })
    return in_maps


def kernel(x, g, alpha, beta):
    from concourse.bass_utils import run_bass_kernel_spmd

    nc = get_module()
    x = np.ascontiguousarray(x, dtype=np.float32)
    g = np.ascontiguousarray(g, dtype=np.float32)
    alpha = np.ascontiguousarray(alpha, dtype=np.float32)
    beta = np.ascontiguousarray(beta, dtype=np.float32)
    in_maps = make_in_maps(x, g, alpha, beta)
    res = run_bass_kernel_spmd(nc, in_maps, list(range(N_CORES)))
    out = np.concatenate([res.results[c]["out"] for c in range(N_CORES)], axis=0)
    return out.astype(np.float32)

